# revision 20
# baseline (speedup 1.0000x reference)
"""Trainium2 Bass kernel for nn_AttentionLayer (sparse/pooled attention).

Reference computation (per batch b, step t):
    q = x @ Wq + bq                          # (N, D)
    k = mean-pool-8(x) @ Wk + bk             # (C, D)   [pool commutes with linear]
    v = mean-pool-8(x) @ Wv + bv             # (C, D)
    per head h (HD=64):
        score = qh @ khT / 8 + adp_pos       # (N, C)
        attn  = softmax(score, axis=-1)
        outh  = attn @ vh                    # (N, HD)
    y = concat(outh) @ Wo + bo               # (N, D)

Sharding: data-parallel over batch B=16 -> 2 per NeuronCore x 8 cores.
All matmuls run in bf16 (f32 PSUM accumulation); verified rel-err ~2e-3.

Key layout choices (per core):
  - xT (d-on-partitions) produced via f32->bf16 cast DMA (SWDGE) to a DRAM
    scratch, then HWDGE xbar DMA-transpose loads.
  - k/v are projected from the 8x-pooled input (pooling done on TensorE with
    a [128,16] averaging matrix), shrinking those projections by 8x.
  - scores are computed in natural [n, c] orientation so the softmax sum and
    reciprocal are per-partition ops; exp(adp_pos) enters multiplicatively.
  - attn rows are re-transposed with xbar DMA for the attn@v matmul; the two
    heads of a pair land in one PSUM tile (col-groups 0-63 / 64-127), giving
    the K=128 stationary tiles the output projection wants.
"""

import os

import numpy as np

B, T, N, D = 16, 12, 1024, 512
H, HD, C = 8, 64, 128
NCORES = 8
BS = B // NCORES          # batch per core
NBT = BS * T              # (b, t) slices per core
MT = N // 128             # m-tiles per (b, t) slice  = 8
CI = D // 128             # contraction chunks        = 4
POOL = N // C             # pooling factor            = 8


def build_kernel(nc, n_bt=NBT):
    """Emit the full per-core kernel graph into `nc` (a bacc.Bacc)."""
    import concourse.bass as bass
    import concourse.tile as tile
    from concourse import mybir

    f32 = mybir.dt.float32
    bf16 = mybir.dt.bfloat16
    AF = mybir.ActivationFunctionType
    ALU = mybir.AluOpType

    M = n_bt * N

    x_in = nc.dram_tensor("x", [BS, T, N, D], f32, kind="ExternalInput").ap()
    Wq_in = nc.dram_tensor("Wq", [D, D], f32, kind="ExternalInput").ap()
    bq_in = nc.dram_tensor("bq", [D], f32, kind="ExternalInput").ap()
    Wk_in = nc.dram_tensor("Wk", [D, D], f32, kind="ExternalInput").ap()
    bk_in = nc.dram_tensor("bk", [D], f32, kind="ExternalInput").ap()
    Wv_in = nc.dram_tensor("Wv", [D, D], f32, kind="ExternalInput").ap()
    bv_in = nc.dram_tensor("bv", [D], f32, kind="ExternalInput").ap()
    Wo_in = nc.dram_tensor("Wo", [D, D], f32, kind="ExternalInput").ap()
    bo_in = nc.dram_tensor("bo", [D], f32, kind="ExternalInput").ap()
    adp_in = nc.dram_tensor("adp_pos", [N, C], f32, kind="ExternalInput").ap()
    y_out = nc.dram_tensor("out", [BS, T, N, D], f32, kind="ExternalOutput").ap()

    x_flat = x_in.rearrange("b t n d -> (b t n) d")
    y_flat = y_out.rearrange("b t n d -> (b t n) d")

    with tile.TileContext(nc) as tc:
        with (
            tc.tile_pool(name="const", bufs=1) as const_pool,
            tc.tile_pool(name="dram", bufs=1, space="DRAM") as dram_pool,
            tc.tile_pool(name="xnat", bufs=2) as xnat_pool,
            tc.tile_pool(name="xt", bufs=2) as xt_pool,
            tc.tile_pool(name="qt", bufs=2) as qt_pool,
            tc.tile_pool(name="pooled", bufs=3) as pooled_pool,
            tc.tile_pool(name="attn", bufs=3) as attn_pool,
            tc.tile_pool(name="outt", bufs=2) as outt_pool,
            tc.tile_pool(name="ysb", bufs=3) as y_pool,
            tc.tile_pool(name="ps", bufs=8, space="PSUM") as ps_pool,
        ):
            # ---------------- constants / weights preload ----------------
            # W* layout: [128, ci*512 + dout] = W[ci*128 + p, dout]  (bf16)
            w_sb = {}
            for nm, w_ap in (("q", Wq_in), ("k", Wk_in), ("v", Wv_in), ("o", Wo_in)):
                w_t = const_pool.tile([128, CI * D], bf16, name=f"W{nm}_sb")
                nc.gpsimd.dma_start(
                    out=w_t[:].rearrange("p (ci dout) -> p ci dout", ci=CI),
                    in_=w_ap.rearrange("(ci p) dout -> p ci dout", p=128),
                )
                w_sb[nm] = w_t

            # per-partition bias tiles [128, dt] for the transposed-q/k evac
            bq_sb = const_pool.tile([128, CI], f32, name="bq_sb")
            nc.sync.dma_start(out=bq_sb[:], in_=bq_in.rearrange("(dt p) -> p dt", p=128))
            bk_sb = const_pool.tile([128, CI], f32, name="bk_sb")
            nc.sync.dma_start(out=bk_sb[:], in_=bk_in.rearrange("(dt p) -> p dt", p=128))

            bv_row = const_pool.tile([1, D], bf16, name="bv_row")
            nc.gpsimd.dma_start(out=bv_row[:], in_=bv_in.unsqueeze(0))
            bo_row = const_pool.tile([1, D], f32, name="bo_row")
            nc.sync.dma_start(out=bo_row[:], in_=bo_in.unsqueeze(0))

            ones_k1 = const_pool.tile([1, 128], bf16, name="ones_k1")
            nc.vector.memset(ones_k1[:], 1.0)
            ones_f32 = const_pool.tile([1, 128], f32, name="ones_f32")
            nc.vector.memset(ones_f32[:], 1.0)

            # bo broadcast to [128, D] via K=1 f32 matmul (one-time)
            bo_bc = const_pool.tile([128, D], f32, name="bo_bc")
            ps_bo = ps_pool.tile([128, D], f32, name="ps_bo", tag="oy", bufs=2)
            nc.tensor.matmul(ps_bo[:], ones_f32[:], bo_row[:], start=True, stop=True)
            nc.scalar.copy(bo_bc[:], ps_bo[:])

            # averaging matrix [128, 16]: pool16[m, j] = 1/8 if m//8 == j,
            # built with two affine_select masks (m-8j >= 0 and m-8j-7 <= 0)
            NJ = 128 // POOL
            pool16 = const_pool.tile([128, NJ], bf16, name="pool16")
            nc.vector.memset(pool16[:], 1.0 / POOL)
            nc.gpsimd.affine_select(
                pool16[:], pool16[:], pattern=[[-POOL, NJ]],
                compare_op=mybir.AluOpType.is_ge, fill=0.0,
                base=0, channel_multiplier=1,
            )
            nc.gpsimd.affine_select(
                pool16[:], pool16[:], pattern=[[POOL, NJ]],
                compare_op=mybir.AluOpType.is_ge, fill=0.0,
                base=POOL - 1, channel_multiplier=-1,
            )

            # exp(adp_pos), natural layout: [128, nt*128 + c] (bf16)
            adp_f = const_pool.tile([128, MT * C], f32, name="adp_f")
            nc.sync.dma_start(
                out=adp_f[:].rearrange("p (nt c) -> p nt c", nt=MT),
                in_=adp_in.rearrange("(nt p) c -> p nt c", p=128),
            )
            eadp = const_pool.tile([128, MT * C], bf16, name="eadp")
            nc.scalar.activation(eadp[:], adp_f[:], AF.Exp)

            # bf16 copy of x in DRAM (written back from SBUF; feeds the
            # xbar transposes, which need a 2-byte DRAM source)
            x16 = dram_pool.tile([M, D], bf16, name="x16")

            # ---------------- main loop over (b, t) slices ----------------
            # Software-pipelined: input staging runs one slice ahead; the
            # attn@v + output projection (which wait on the big attn
            # transpose) run one slice behind, hiding the transpose latency
            # behind the next slice's score/softmax work.

            def stage_inputs(bt):
                """cast-load x, spill bf16 copy, xbar-transpose -> (x_nat, xT)"""
                r0 = bt * N
                x_nat = xnat_pool.tile([128, MT * D], bf16, name="x_nat",
                                       tag="x_nat")
                nc.gpsimd.dma_start(
                    out=x_nat[:].rearrange("p (mt d) -> p mt d", mt=MT),
                    in_=x_flat[r0:r0 + N, :].rearrange("(mt p) d -> p mt d", p=128),
                )
                nc.gpsimd.dma_start(
                    out=x16[r0:r0 + N, :].rearrange("(mt p) d -> p mt d", p=128),
                    in_=x_nat[:].rearrange("p (mt d) -> p mt d", mt=MT),
                )
                xT = xt_pool.tile([128, CI * N], bf16, name="xT", tag="xT")
                for ci in range(CI):
                    nc.sync.dma_start(
                        out=xT[:, ci * N:(ci + 1) * N],
                        in_=x16[r0:r0 + N, ci * 128:(ci + 1) * 128],
                        transpose=True,
                    )
                return x_nat, xT

            def phase_c_pair(vp, attnT_all, outT, pp):
                """attn @ v for one head pair (pair-stacked psum)"""
                heads = (2 * pp, 2 * pp + 1)
                ps_o = [
                    ps_pool.tile([128, 512], f32, name=f"ps_o{half}",
                                 tag="oy", bufs=2)
                    for half in range(2)
                ]
                for half in range(2):
                    for h in heads:
                        nc.tensor.matmul(
                            ps_o[half][(h % 2) * 64:(h % 2) * 64 + 64, :],
                            vp[:, h * 64:(h + 1) * 64],
                            attnT_all[:, h * N + half * 512: h * N + (half + 1) * 512],
                            start=True,
                            stop=True,
                        )
                for half in range(2):
                    nc.scalar.copy(
                        outT[:, pp * N + half * 512: pp * N + (half + 1) * 512],
                        ps_o[half][:],
                    )

            def phase_d(outT, r0):
                """output projection + bias for one slice"""
                for mt in range(MT):
                    ps_y = ps_pool.tile([128, 512], f32, name="ps_y",
                                        tag="oy", bufs=2)
                    for pp in range(4):
                        nc.tensor.matmul(
                            ps_y[:],
                            outT[:, pp * N + mt * 128: pp * N + (mt + 1) * 128],
                            w_sb["o"][:, pp * D:(pp + 1) * D],
                            start=(pp == 0),
                            stop=(pp == 3),
                        )
                    y_sb = y_pool.tile([128, D], f32, name="y_sb")
                    nc.vector.tensor_tensor(
                        y_sb[:], ps_y[:], bo_bc[:], op=ALU.add
                    )
                    nc.gpsimd.dma_start(
                        out=y_flat[r0 + mt * 128: r0 + (mt + 1) * 128, :],
                        in_=y_sb[:],
                    )

            staged = stage_inputs(0)
            pending = []
            for bt in range(n_bt):
                r0 = bt * N
                x_nat, xT = staged
                if bt + 1 < n_bt:
                    staged = stage_inputs(bt + 1)

                # slice two iterations back: its attn@v pairs are woven
                # into this slice's score/softmax phase below so the PE
                # keeps a dense stream of K=128 matmuls (HAM stays warm)
                cd = pending.pop(0) if len(pending) >= 2 else None
                cd_outT = (outt_pool.tile([128, CI * N], bf16, name="cd_outT")
                           if cd is not None else None)

                # ---- q projection (transposed out): qT[p, dt*1024+m] ----
                qT = qt_pool.tile([128, CI * N], bf16, name="qT")
                for dt in range(CI):
                    for mc in range(N // 512):
                        ps_q = ps_pool.tile([128, 512], f32, name="ps_q", tag="proj", bufs=2)
                        for ci in range(CI):
                            nc.tensor.matmul(
                                ps_q[:],
                                w_sb["q"][:, ci * D + dt * 128: ci * D + dt * 128 + 128],
                                xT[:, ci * N + mc * 512: ci * N + mc * 512 + 512],
                                start=(ci == 0),
                                stop=(ci == CI - 1),
                            )
                        nc.scalar.activation(
                            qT[:, dt * N + mc * 512: dt * N + mc * 512 + 512],
                            ps_q[:],
                            AF.Identity,
                            bias=bq_sb[:, dt:dt + 1],
                        )

                # ---- pooled input (transposed): xpT[p, ci*128 + c] ----
                ps_xp = ps_pool.tile([128, 512], f32, name="ps_xp", tag="proj", bufs=2)
                for mt in range(MT):
                    for ci in range(CI):
                        nc.tensor.matmul(
                            ps_xp[:, ci * 128 + mt * 16: ci * 128 + (mt + 1) * 16],
                            x_nat[:, mt * D + ci * 128: mt * D + ci * 128 + 128],
                            pool16[:],
                            start=True,
                            stop=True,
                        )
                xpT = pooled_pool.tile([128, CI * C], bf16, name="xpT")
                nc.vector.tensor_copy(xpT[:], ps_xp[:])

                # ---- pooled k projection (transposed): kpT[p, dt*128+c] ----
                ps_kp = ps_pool.tile([128, 512], f32, name="ps_kp", tag="proj", bufs=2)
                for dt in range(CI):
                    for ci in range(CI):
                        nc.tensor.matmul(
                            ps_kp[:, dt * 128:(dt + 1) * 128],
                            w_sb["k"][:, ci * D + dt * 128: ci * D + dt * 128 + 128],
                            xpT[:, ci * C:(ci + 1) * C],
                            start=(ci == 0),
                            stop=(ci == CI - 1),
                        )
                kpT = pooled_pool.tile([128, CI * C], bf16, name="kpT")
                for dt in range(CI):
                    nc.scalar.activation(
                        kpT[:, dt * 128:(dt + 1) * 128],
                        ps_kp[:, dt * 128:(dt + 1) * 128],
                        AF.Identity,
                        bias=bk_sb[:, dt:dt + 1],
                    )

                # ---- pooled v projection (natural): vp[c, dout] ----
                ps_vp = ps_pool.tile([128, 512], f32, name="ps_vp", tag="proj", bufs=2)
                for ci in range(CI):
                    nc.tensor.matmul(
                        ps_vp[:],
                        xpT[:, ci * C:(ci + 1) * C],
                        w_sb["v"][:, ci * D:(ci + 1) * D],
                        start=(ci == 0),
                        stop=False,
                    )
                nc.tensor.matmul(
                    ps_vp[:], ones_k1[:], bv_row[:], start=False, stop=True
                )
                vp = pooled_pool.tile([128, D], bf16, name="vp")
                nc.scalar.copy(vp[:], ps_vp[:])

                # ---- attention phase A: softmax rows for all 8 heads ----
                # (pair-interleaved scores so K=64 matmuls pack onto disjoint
                # PE row-groups); normalized rows land in one DRAM buffer
                attn_dr = dram_pool.tile([H * N, C], bf16, name="attn_dr",
                                         tag="attn_dr", bufs=3)
                for pp in range(4):
                    if cd is not None:
                        phase_c_pair(cd[0], cd[1], cd_outT, pp)
                    heads = (2 * pp, 2 * pp + 1)
                    phs = {h: (h % 2) * 64 for h in heads}

                    # scores, natural [n, c]; one 2-bank psum per head
                    ps_sc = {
                        h: ps_pool.tile([128, MT * C], f32,
                                        name=f"ps_sc{h % 2}", tag="sc", bufs=2)
                        for h in heads
                    }
                    for nt in range(MT):
                        for h in heads:
                            ph = phs[h]
                            nc.tensor.matmul(
                                ps_sc[h][:, nt * C:(nt + 1) * C],
                                qT[ph:ph + 64, pp * N + nt * 128: pp * N + (nt + 1) * 128],
                                kpT[ph:ph + 64, pp * 128:(pp + 1) * 128],
                                start=True,
                                stop=True,
                            )

                    for h in heads:
                        par = h % 2
                        exp_sb = attn_pool.tile([128, MT * C], bf16,
                                                name=f"exp_sb{par}")
                        nc.scalar.activation(
                            exp_sb[:], ps_sc[h][:], AF.Exp,
                            scale=1.0 / np.sqrt(HD),
                        )

                        # u = exp * exp(adp);  s[n] = sum_c u;  r = 1/s (bf16)
                        u_sb = attn_pool.tile([128, MT * C], bf16, name=f"u_sb{par}")
                        nc.vector.tensor_tensor(u_sb[:], exp_sb[:], eadp[:],
                                                op=ALU.mult)
                        s_sb = attn_pool.tile([128, MT], bf16, name=f"s_sb{par}")
                        r_sb = attn_pool.tile([128, MT], bf16, name=f"r_sb{par}")
                        with nc.allow_low_precision("softmax denom, ~0.4% ok"):
                            nc.vector.reduce_sum(
                                s_sb[:],
                                u_sb[:].rearrange("p (nt c) -> p nt c", nt=MT),
                                axis=mybir.AxisListType.X,
                            )
                            nc.vector.reciprocal(r_sb[:], s_sb[:])

                        # attn = u * r (stride-0 broadcast of r along c)
                        attn_sb = attn_pool.tile([128, MT * C], bf16,
                                                 name=f"attn_sb{par}")
                        nc.vector.tensor_tensor(
                            attn_sb[:].rearrange("p (nt c) -> p nt c", nt=MT),
                            u_sb[:].rearrange("p (nt c) -> p nt c", nt=MT),
                            r_sb[:].unsqueeze(2).broadcast_to((128, MT, C)),
                            op=ALU.mult,
                        )
                        nc.gpsimd.dma_start(
                            out=attn_dr[h * N:(h + 1) * N, :].rearrange(
                                "(nt p) c -> p nt c", p=128),
                            in_=attn_sb[:].rearrange("p (nt c) -> p nt c", nt=MT),
                        )

                if cd is not None:
                    phase_d(cd_outT, cd[2])

                # ---- phase B: one big xbar transpose for all heads ----
                attnT_all = attn_pool.tile([128, H * N], bf16, name="attnT_all",
                                           bufs=3)
                nc.sync.dma_start(out=attnT_all[:], in_=attn_dr[:],
                                  transpose=True)

                pending.append((vp, attnT_all, r0))

            for args in pending:
                tail_outT = outt_pool.tile([128, CI * N], bf16, name="tail_outT")
                for pp in range(4):
                    phase_c_pair(args[0], args[1], tail_outT, pp)
                phase_d(tail_outT, args[2])

    return nc


_COMPILED = {}


def _get_compiled(n_bt=NBT):
    if n_bt not in _COMPILED:
        from concourse import bacc

        nc = bacc.Bacc("TRN2", target_bir_lowering=False, debug=False,
                       num_devices=NCORES)
        build_kernel(nc, n_bt)
        nc.compile()
        _COMPILED[n_bt] = nc
    return _COMPILED[n_bt]


def kernel(**inputs):
    """Full-input entry point: shards over batch across 8 cores."""
    os.environ.setdefault("JAX_PLATFORMS", "axon,cpu")
    os.environ.setdefault("NEURON_RT_RESET_CORES", "1")
    from concourse.bass_utils import run_bass_kernel_spmd

    nc = _get_compiled()

    x = np.ascontiguousarray(inputs["x"], dtype=np.float32)
    params = {
        k: np.ascontiguousarray(inputs[k], dtype=np.float32)
        for k in ("Wq", "bq", "Wk", "bk", "Wv", "bv", "Wo", "bo", "adp_pos")
    }
    in_maps = []
    for core in range(NCORES):
        m = {"x": x[core * BS:(core + 1) * BS]}
        m.update(params)
        in_maps.append(m)

    res = run_bass_kernel_spmd(nc, in_maps, core_ids=list(range(NCORES)))
    out = np.concatenate([res.results[i]["out"] for i in range(NCORES)], axis=0)
    return out


if __name__ == "__main__":
    import jax

    jax.config.update("jax_platforms", "cpu")
    import reference

    inputs = reference.setup_inputs()
    inputs = {k: np.asarray(v) for k, v in inputs.items()}
    expected = np.asarray(reference.reference(**inputs))
    actual = kernel(**inputs)
    err = np.linalg.norm(actual - expected) / np.linalg.norm(expected)
    print("Relative error:", err)
